# revision 1
# baseline (speedup 1.0000x reference)
"""Trainium2 Bass kernel for retrieval_knn (nn_CLI_v1_63702954934484).

Reference computation (per batch b):
    dist[n,m] = ||ca[n] - cb[m]|| / 128                         [Na, Nb]
    idx       = argtop4-smallest(dist[n,:])                     [Na, 4]
    dw        = R - clip(dist_top4, 0, R)                       [Na, 4]
    h         = [b_f, a_f - b_f]  (b_f = feats_b[idx])          [Na, 4, 2D]
    fused     = sum_k relu(h @ W + bias) * dw                   [Na, D]
    out       = [feats_a, fused]                                [Na, 2D]

Kernel restructure (exact up to fp32 rounding):
  * h @ W + bias = a_f @ W2 + b_f @ (W1 - W2) + bias
    so precompute Ya = feats_a @ W2 + bias and Yb = feats_b @ (W1 - W2)
    once per batch and GATHER ROWS OF Yb (same cost as gathering feats_b
    but 4x fewer matmul FLOPs).
  * dw >= 0, so dw * relu(z) = relu(dw * z): the weighting folds into the
    scalar-engine activation's per-partition scale operand.
  * Distances: -dist2 (integer-exact in fp32) via a single K=5 matmul of
    lifted coords  phia = [a0,a1,a2,|a|^2,1], phib = [2b0,2b1,2b2,-1,-|b|^2].
    Ordering by -dist2 == ordering by dist; ties break identically to
    jax.lax.top_k (hw max_index assigns ascending indices to duplicates).
  * top-4 via DVE max8 + max_index straight out of PSUM.
  * neighbor rows fetched with an indirect DMA gather (compute_op=add)
    that accumulates Yb rows onto an SBUF tile prefilled with Ya, giving
    z = Ya + Yb[idx] for free.

Sharding: data-parallel over batch (16 batches -> 8 cores x 2).
"""

import sys

sys.path.insert(0, "/opt/trn_rl_repo")

import numpy as np

import concourse.bass as bass
import concourse.mybir as mybir
import concourse.tile as tile
from concourse import bacc
from concourse.bass import IndirectOffsetOnAxis
from concourse.masks import make_identity

F32 = mybir.dt.float32
U32 = mybir.dt.uint32

P = 128          # partitions
D = 512          # feature dim
KNN = 4          # neighbors
R = 0.5
FULL_SCALE = 128.0

B = 16           # full batch
N_CORES = 8
BLOC = B // N_CORES  # batches per core

NA = 2048
NB = 2048

# knobs
USE_DMA_ADD = True     # fuse z = Ya + Yb[idx] into the gather DMA
# number of indirect DMA instructions per n-tile. MUST be KNN: multi-offset
# indirect DMAs ([128, >1] offset APs) crash the device (NRT unrecoverable),
# single-offset [128, 1] gathers are solid.
GATHER_SPLIT = 4

DEBUG_DUMP = False  # debug builds add intermediate-dump outputs (batch 0)
GATHER_BOUNDS_CHECK = False  # debug: error on OOB gather indices


def build_bass(bloc=BLOC, na=NA, nb=NB, enable_asserts=False):
    """Build the per-core Bass program. Same program runs on all 8 cores.

    Built on Bacc so compile() runs generate_event_semaphores, which splits
    multi-sem waits (walrus allows at most 1 wait/instruction, 2 on EVSEM).
    """
    nc = bacc.Bacc("TRN2", debug=False, enable_asserts=enable_asserts)
    nt = na // P          # n-tiles
    dt = D // P           # 128-chunks of the feature dim
    ncs = min(512, nb)    # candidate-dim chunk (PSUM bank = 512 fp32)
    nbt = nb // ncs       # chunks of the candidate dim

    featsa = nc.dram_tensor("featsa", [bloc, na, D], F32, kind="ExternalInput").ap()
    featsb = nc.dram_tensor("featsb", [bloc, nb, D], F32, kind="ExternalInput").ap()
    phiat = nc.dram_tensor("phiat", [bloc, 5, na], F32, kind="ExternalInput").ap()
    phibt = nc.dram_tensor("phibt", [bloc, 5, nb], F32, kind="ExternalInput").ap()
    w2b = nc.dram_tensor("w2b", [D + 1, D], F32, kind="ExternalInput").ap()
    wd = nc.dram_tensor("wd", [D, D], F32, kind="ExternalInput").ap()
    out = nc.dram_tensor("out", [bloc, na, 2 * D], F32, kind="ExternalOutput").ap()

    dbg = None
    if DEBUG_DUMP:
        dbg = {
            "yb2": nc.dram_tensor("dbg_yb2", [nb, D], F32, kind="ExternalOutput").ap(),
            "negd": nc.dram_tensor("dbg_negd", [nt, P, 8], F32, kind="ExternalOutput").ap(),
            "idx": nc.dram_tensor("dbg_idx", [nt, P, 8], U32, kind="ExternalOutput").ap(),
            "dw": nc.dram_tensor("dbg_dw", [nt, P, KNN], F32, kind="ExternalOutput").ap(),
            "yb": nc.dram_tensor("dbg_yb", [nb, D], F32, kind="ExternalOutput").ap(),
            "z": nc.dram_tensor("dbg_z", [P, KNN, D], F32, kind="ExternalOutput").ap(),
        }

    with tile.TileContext(nc) as tc:
        _kern(tc, featsa, featsb, phiat, phibt, w2b, wd, out,
              bloc=bloc, na=na, nb=nb, nt=nt, dt=dt, nbt=nbt, ncs=ncs, dbg=dbg)
    nc.compile()
    return nc


def _kern(tc, featsa, featsb, phiat, phibt, w2b, wd, out, *, bloc, na, nb, nt, dt, nbt, ncs, dbg=None):
    nc = tc.nc
    with (
        tc.tile_pool(name="const", bufs=1) as cpool,
        tc.tile_pool(name="wpool", bufs=1) as wpool,
        tc.tile_pool(name="phi", bufs=2) as phipool,
        tc.tile_pool(name="io", bufs=3) as iopool,
        tc.tile_pool(name="tr", bufs=3) as trpool,
        tc.tile_pool(name="stage", bufs=3) as stpool,
        tc.tile_pool(name="topk", bufs=1) as tkpool,
        tc.tile_pool(name="gat", bufs=3) as gpool,
        tc.tile_pool(name="mlp", bufs=2) as mpool,
        tc.tile_pool(name="dram", bufs=2, space="DRAM") as dpool,
    ):
        ident = cpool.tile([P, P], F32, name="ident")
        make_identity(nc, ident)
        ones_t = cpool.tile([1, P], F32, name="ones_t")
        nc.vector.memset(ones_t, 1.0)
        rconst = cpool.tile([P, 1], F32, name="rconst")
        nc.vector.memset(rconst, R)
        zconst = cpool.tile([P, 1], F32, name="zconst")
        nc.vector.memset(zconst, 0.0)

        # resident weights: w2 chunks [128, j, 512], bias row, wd chunks
        w2_sb = wpool.tile([P, dt, D], F32, name="w2_sb")
        wd_sb = wpool.tile([P, dt, D], F32, name="wd_sb")
        bias_sb = wpool.tile([1, D], F32, name="bias_sb")
        for j in range(dt):
            nc.sync.dma_start(out=w2_sb[:, j, :], in_=w2b[j * P:(j + 1) * P, :])
            nc.sync.dma_start(out=wd_sb[:, j, :], in_=wd[j * P:(j + 1) * P, :])
        nc.sync.dma_start(out=bias_sb, in_=w2b[D:D + 1, :])

        for b in range(bloc):
            # ---------------- stage D: distances + top-k ----------------
            phia_sb = phipool.tile([5, na], F32, tag="phia", name="phia_sb")
            phib_sb = phipool.tile([5, nb], F32, tag="phib", name="phib_sb")
            nc.sync.dma_start(out=phia_sb, in_=phiat[b])
            nc.sync.dma_start(out=phib_sb, in_=phibt[b])

            negd_t = []
            idx_t = []
            dw_t = []
            with tc.tile_pool(name="dist_ps", bufs=2, space="PSUM") as dps:
                for i in range(nt):
                    dist_ps = dps.tile([P, nb], F32, tag="dist", name="dist_ps")
                    for j in range(nbt):
                        nc.tensor.matmul(
                            out=dist_ps[:, j * ncs:(j + 1) * ncs],
                            lhsT=phia_sb[:, i * P:(i + 1) * P],
                            rhs=phib_sb[:, j * ncs:(j + 1) * ncs],
                            start=True, stop=True,
                        )
                    negd = tkpool.tile([P, 8], F32, tag=f"negd{i}", name="negd")
                    nc.vector.max(out=negd, in_=dist_ps)
                    idx = tkpool.tile([P, 8], U32, tag=f"idx{i}", name="idx")
                    nc.vector.max_index(out=idx, in_max=negd, in_values=dist_ps)
                    # dist = sqrt(-negd); dw = relu(R - dist/FULL_SCALE)
                    dist4 = stpool.tile([P, KNN], F32, tag="dist4", name="dist4")
                    nc.scalar.activation(
                        out=dist4, in_=negd[:, :KNN],
                        func=mybir.ActivationFunctionType.Sqrt, scale=-1.0,
                        bias=zconst[:, :1])
                    dw = tkpool.tile([P, KNN], F32, tag=f"dw{i}", name="dw")
                    nc.scalar.activation(
                        out=dw, in_=dist4,
                        func=mybir.ActivationFunctionType.Relu,
                        scale=-1.0 / FULL_SCALE, bias=rconst[:, :1])
                    negd_t.append(negd)
                    idx_t.append(idx)
                    dw_t.append(dw)
                    if dbg is not None and b == 0:
                        nc.sync.dma_start(out=dbg["negd"][i], in_=negd)
                        nc.sync.dma_start(out=dbg["idx"][i], in_=idx)
                        nc.sync.dma_start(out=dbg["dw"][i], in_=dw)

            yb_dram = dpool.tile([nb, D], F32, tag="ybd", name="yb_dram")

            with (
                tc.tile_pool(name="tp_ps", bufs=2, space="PSUM") as tpps,
                tc.tile_pool(name="mm_ps", bufs=2, space="PSUM") as mmps,
            ):
                # ------------- stage B-side: Yb = feats_b @ Wd -> DRAM -------------
                for i in range(nt):
                    fb = iopool.tile([P, D], F32, tag="fb", name="fb")
                    nc.sync.dma_start(out=fb, in_=featsb[b, i * P:(i + 1) * P, :])
                    yb_ps = mmps.tile([P, D], F32, tag="mm", name="yb_ps")
                    for j in range(dt):
                        tp_ps = tpps.tile([P, P], F32, tag="tp", name="tp_ps")
                        nc.tensor.transpose(out=tp_ps, in_=fb[:, j * P:(j + 1) * P],
                                            identity=ident)
                        bt = trpool.tile([P, P], F32, tag="bt", name="bt")
                        nc.scalar.copy(out=bt, in_=tp_ps)
                        nc.tensor.matmul(out=yb_ps, lhsT=bt, rhs=wd_sb[:, j, :],
                                         start=(j == 0), stop=(j == dt - 1))
                    ybst = stpool.tile([P, D], F32, tag="ybst", name="ybst")
                    nc.vector.tensor_copy(out=ybst, in_=yb_ps)
                    nc.sync.dma_start(out=yb_dram[i * P:(i + 1) * P, :], in_=ybst)
                    if dbg is not None and b == 0:
                        nc.sync.dma_start(out=dbg["yb"][i * P:(i + 1) * P, :], in_=ybst)

                # ------------- stage A-side + gather + MLP -------------
                for i in range(nt):
                    fa = iopool.tile([P, D], F32, tag="fa", name="fa")
                    nc.sync.dma_start(out=fa, in_=featsa[b, i * P:(i + 1) * P, :])
                    # pass feats_a through to the left half of the output
                    nc.sync.dma_start(out=out[b, i * P:(i + 1) * P, 0:D], in_=fa)

                    ya_ps = mmps.tile([P, D], F32, tag="mm", name="ya_ps")
                    for j in range(dt):
                        tp_ps = tpps.tile([P, P], F32, tag="tp", name="tp_ps")
                        nc.tensor.transpose(out=tp_ps, in_=fa[:, j * P:(j + 1) * P],
                                            identity=ident)
                        at = trpool.tile([P, P], F32, tag="at", name="at")
                        nc.scalar.copy(out=at, in_=tp_ps)
                        nc.tensor.matmul(out=ya_ps, lhsT=at, rhs=w2_sb[:, j, :],
                                         start=(j == 0), stop=False)
                    nc.tensor.matmul(out=ya_ps, lhsT=ones_t, rhs=bias_sb,
                                     start=False, stop=True)

                    # four separate full-tile gather destinations (sliced
                    # dst APs / fused compute-add were implicated in HW-only
                    # corruption; plain full-tile gathers are proven solid)
                    idx = idx_t[i]
                    ybg_k = []
                    for k in range(KNN):
                        ybg = gpool.tile([P, D], F32, tag=f"ybg{k}", name=f"ybg{k}")
                        nc.gpsimd.indirect_dma_start(
                            out=ybg[:],
                            out_offset=None,
                            in_=yb_dram[:],
                            in_offset=IndirectOffsetOnAxis(
                                ap=idx[:, k:k + 1], axis=0),
                        )
                        ybg_k.append(ybg)
                    ya_sb = stpool.tile([P, D], F32, tag="ya_sb", name="ya_sb")
                    nc.vector.tensor_copy(out=ya_sb, in_=ya_ps)
                    z_k = []
                    for k in range(KNN):
                        zk = mpool.tile([P, D], F32, tag=f"z{k}", name=f"z{k}")
                        nc.vector.tensor_add(zk, ybg_k[k], ya_sb)
                        z_k.append(zk)

                    if dbg is not None and b == 0 and i == 0:
                        for k in range(KNN):
                            nc.sync.dma_start(out=dbg["z"][:, k, :], in_=z_k[k])
                    if dbg is not None and b == 0 and i == nt - 1:
                        # read the scratch back from DRAM through SBUF
                        for i2 in range(nt):
                            ybrb = stpool.tile([P, D], F32, tag="ybrb", name="ybrb")
                            nc.sync.dma_start(out=ybrb, in_=yb_dram[i2 * P:(i2 + 1) * P, :])
                            nc.sync.dma_start(out=dbg["yb2"][i2 * P:(i2 + 1) * P, :], in_=ybrb)
                    # r_k = relu(dw_k * z_k) == dw_k * relu(z_k); write into
                    # the (now free) gather tiles, no in-place ops
                    dw = dw_t[i]
                    for k in range(KNN):
                        nc.scalar.activation(
                            out=ybg_k[k][:], in_=z_k[k][:],
                            func=mybir.ActivationFunctionType.Relu,
                            scale=dw[:, k:k + 1])
                    t01 = mpool.tile([P, D], F32, tag="t01", name="t01")
                    nc.vector.tensor_add(t01, ybg_k[0], ybg_k[1])
                    t23 = mpool.tile([P, D], F32, tag="t23", name="t23")
                    nc.vector.tensor_add(t23, ybg_k[2], ybg_k[3])
                    fused = mpool.tile([P, D], F32, tag="fused", name="fused")
                    nc.vector.tensor_add(fused, t01, t23)
                    nc.sync.dma_start(out=out[b, i * P:(i + 1) * P, D:2 * D], in_=fused)


# ---------------------------------------------------------------------------
# host side
# ---------------------------------------------------------------------------

def _host_inputs(feats_a, feats_b, W, bias, coords_a, coords_b):
    """Precompute the tiny host-side tensors (weight split, lifted coords)."""
    nb_, d_ = W.shape[0] // 2, W.shape[1]
    ca = coords_a.astype(np.float32)
    cb = coords_b.astype(np.float32)
    bsz = ca.shape[0]
    # phia = [a0,a1,a2,|a|^2,1] ; phib = [2b0,2b1,2b2,-1,-|b|^2]
    # => phia . phib = 2 a.b - |a|^2 - |b|^2 = -dist2 (exact small ints)
    phia = np.concatenate(
        [ca, (ca * ca).sum(-1, keepdims=True),
         np.ones((bsz, ca.shape[1], 1), np.float32)], axis=-1)
    phib = np.concatenate(
        [2.0 * cb, -np.ones((bsz, cb.shape[1], 1), np.float32),
         -(cb * cb).sum(-1, keepdims=True)], axis=-1)
    phiaT = np.ascontiguousarray(phia.transpose(0, 2, 1))
    phibT = np.ascontiguousarray(phib.transpose(0, 2, 1))
    w2 = W[nb_:]                      # applies to a_f
    wdm = np.ascontiguousarray(W[:nb_] - W[nb_:])   # applies to b_f
    w2b = np.concatenate([w2, bias[None, :].astype(np.float32)], axis=0)
    return phiaT, phibT, np.ascontiguousarray(w2b), wdm


def kernel(**inputs):
    feats_a = np.ascontiguousarray(np.asarray(inputs["feats_a"], dtype=np.float32))
    feats_b = np.ascontiguousarray(np.asarray(inputs["feats_b"], dtype=np.float32))
    W = np.asarray(inputs["W"], dtype=np.float32)
    bias = np.asarray(inputs["bias"], dtype=np.float32)
    coords_a = np.asarray(inputs["coords_a"])
    coords_b = np.asarray(inputs["coords_b"])

    phiaT, phibT, w2b, wdm = _host_inputs(feats_a, feats_b, W, bias,
                                          coords_a, coords_b)

    nc = build_bass()

    in_maps = []
    for c in range(N_CORES):
        s = slice(c * BLOC, (c + 1) * BLOC)
        in_maps.append({
            "featsa": feats_a[s],
            "featsb": feats_b[s],
            "phiat": phiaT[s],
            "phibt": phibT[s],
            "w2b": w2b,
            "wd": wdm,
        })

    from concourse import bass_utils
    res = bass_utils.run_bass_kernel_spmd(nc, in_maps, core_ids=list(range(N_CORES)))
    outs = [r["out"] for r in res.results]
    return np.concatenate(outs, axis=0)


if __name__ == "__main__":
    nc = build_bass()
    print("built ok")



# revision 2
# speedup vs baseline: 2.1601x; 2.1601x over previous
"""Trainium2 Bass kernel for retrieval_knn (nn_CLI_v1_63702954934484).

Reference computation (per batch b):
    dist[n,m] = ||ca[n] - cb[m]|| / 128                         [Na, Nb]
    idx       = argtop4-smallest(dist[n,:])                     [Na, 4]
    dw        = R - clip(dist_top4, 0, R)                       [Na, 4]
    h         = [b_f, a_f - b_f]  (b_f = feats_b[idx])          [Na, 4, 2D]
    fused     = sum_k relu(h @ W + bias) * dw                   [Na, D]
    out       = [feats_a, fused]                                [Na, 2D]

Kernel restructure (v2, fp16-centric):
  * h @ W + bias = a_f @ W2 + b_f @ (W1 - W2) + bias. Precompute
    Ya = feats_a @ W2 (+bias) and Yb = feats_b @ (W1-W2) per batch, then
    gather ROWS of Yb. All feature matmuls run in fp16 (1 cyc/row on PE vs
    4 for fp32), with lhsT pre-transposed on the host (no PE transposes).
  * Distances: the matmul computes packed = -(dist2 + m/2048) in one K=8
    fp16 matmul of lifted coords. All lifted values are exactly
    representable in fp16 and all products/sums are exact in fp32
    accumulation (for the small dist2 that can enter the top-4), so the
    ordering is bit-identical to the fp32 reference, ties break by smaller
    index = jax.lax.top_k behavior. A single DVE max8 pass then yields both
    the top-4 distances AND the neighbor indices (unpacked arithmetically)
    -- no second max_index scan.
  * top-4 via max8 on two 1024-wide PSUM halves + an 8+8 -> 8 merge.
  * fused output is written fp16 (well within the 2e-2 gate); the host
    upcasts and concatenates the feats_a passthrough half (never touches
    the device).

Sharding: data-parallel over batch (16 batches -> 8 cores x 2).
"""

import sys

sys.path.insert(0, "/opt/trn_rl_repo")

import numpy as np

import concourse.bass as bass
import concourse.mybir as mybir
import concourse.tile as tile
from concourse import bacc
from concourse.bass import IndirectOffsetOnAxis

F32 = mybir.dt.float32
F16 = mybir.dt.float16
U32 = mybir.dt.uint32

P = 128          # partitions
D = 512          # feature dim
KNN = 4          # neighbors
R = 0.5
FULL_SCALE = 128.0

B = 16           # full batch
N_CORES = 8
BLOC = B // N_CORES  # batches per core

NA = 2048
NB = 2048
NT = NA // P     # n-tiles per batch
DT = D // P      # 128-chunks of the feature dim
HALF = 1024      # distance column chunk (2 PSUM banks)

AF = mybir.ActivationFunctionType
ALU = mybir.AluOpType

# how many of the 4 relu*dw ops go to the scalar engine (rest on DVE via
# dual-op tensor_scalar). Tunable for engine balance.
RELU_ON_ACT = 4


def build_bass(bloc=BLOC, na=NA, nb=NB, with_bias=False):
    nc = bacc.Bacc("TRN2", debug=False)
    fatT = nc.dram_tensor("fatT", [bloc, DT, P, na], F16, kind="ExternalInput").ap()
    fbtT = nc.dram_tensor("fbtT", [bloc, DT, P, nb], F16, kind="ExternalInput").ap()
    phia = nc.dram_tensor("phia", [bloc, 8, na], F16, kind="ExternalInput").ap()
    phib = nc.dram_tensor("phib", [bloc, 8, nb], F16, kind="ExternalInput").ap()
    w2 = nc.dram_tensor("w2", [DT, P, D], F16, kind="ExternalInput").ap()
    wd = nc.dram_tensor("wd", [DT, P, D], F16, kind="ExternalInput").ap()
    biasw = nc.dram_tensor("biasw", [1, D], F16, kind="ExternalInput").ap()
    out = nc.dram_tensor("out", [bloc, na, D], F16, kind="ExternalOutput").ap()

    with tile.TileContext(nc) as tc:
        _kern(tc, fatT, fbtT, phia, phib, w2, wd, biasw, out,
              bloc=bloc, na=na, nb=nb, with_bias=with_bias)
    nc.compile()
    return nc


def _kern(tc, fatT, fbtT, phia, phib, w2, wd, biasw, out, *, bloc, na, nb,
          with_bias):
    nc = tc.nc
    nt = na // P
    with (
        tc.tile_pool(name="const", bufs=1) as cpool,
        tc.tile_pool(name="wpool", bufs=1) as wpool,
        tc.tile_pool(name="feat", bufs=2) as fpool,
        tc.tile_pool(name="phi", bufs=2) as phipool,
        tc.tile_pool(name="tk", bufs=2) as tkpool,
        tc.tile_pool(name="ext", bufs=2) as epool,
        tc.tile_pool(name="io", bufs=3) as iopool,
        tc.tile_pool(name="gat", bufs=3) as gpool,
        tc.tile_pool(name="mlp", bufs=2) as mpool,
        tc.tile_pool(name="dram", bufs=2, space="DRAM") as dpool,
    ):
        rconst = cpool.tile([P, 1], F32, name="rconst")
        nc.vector.memset(rconst, R)
        ones_t = cpool.tile([1, P], F16, name="ones_t")
        nc.vector.memset(ones_t, 1.0)

        # resident weights (fp16): w2 / wd chunks [128, j, 512]
        w2_sb = wpool.tile([P, DT, D], F16, name="w2_sb")
        wd_sb = wpool.tile([P, DT, D], F16, name="wd_sb")
        for j in range(DT):
            nc.sync.dma_start(out=w2_sb[:, j, :], in_=w2[j])
            nc.sync.dma_start(out=wd_sb[:, j, :], in_=wd[j])
        bias_sb = wpool.tile([1, D], F16, name="bias_sb")
        if with_bias:
            nc.sync.dma_start(out=bias_sb, in_=biasw)

        for b in range(bloc):
            # ---- per-batch loads ----
            fat_sb = fpool.tile([P, DT, na], F16, tag="fat", name="fat_sb")
            fbt_sb = fpool.tile([P, DT, nb], F16, tag="fbt", name="fbt_sb")
            for j in range(DT):
                nc.sync.dma_start(out=fat_sb[:, j, :], in_=fatT[b, j])
                nc.sync.dma_start(out=fbt_sb[:, j, :], in_=fbtT[b, j])
            phia_sb = phipool.tile([8, na], F16, tag="phia", name="phia_sb")
            phib_sb = phipool.tile([8, nb], F16, tag="phib", name="phib_sb")
            nc.sync.dma_start(out=phia_sb, in_=phia[b])
            nc.sync.dma_start(out=phib_sb, in_=phib[b])

            yb_dram = dpool.tile([nb, D], F16, tag="ybd", name="yb_dram")
            negd = tkpool.tile([P, nt, 8], F32, tag="negd", name="negd")

            with (
                tc.tile_pool(name="dist_ps", bufs=2, space="PSUM") as dps,
                tc.tile_pool(name="mm_ps", bufs=4, space="PSUM") as mmps,
            ):
                # ---- stage 1: Yb (PE+Act) and distances+top8 (PE+DVE) ----
                for i in range(nt):
                    yb_ps = mmps.tile([P, D], F32, tag="mm", name="yb_ps")
                    for j in range(DT):
                        nc.tensor.matmul(
                            out=yb_ps, lhsT=fbt_sb[:, j, i * P:(i + 1) * P],
                            rhs=wd_sb[:, j, :],
                            start=(j == 0), stop=(j == DT - 1))
                    yb_sb = iopool.tile([P, D], F16, tag="ybsb", name="yb_sb")
                    nc.scalar.copy(out=yb_sb, in_=yb_ps)
                    nc.sync.dma_start(out=yb_dram[i * P:(i + 1) * P, :], in_=yb_sb)

                    h16 = epool.tile([P, 16], F32, tag="h16", name="h16")
                    for h in range(2):
                        dist_ps = dps.tile([P, HALF], F32, tag="dist",
                                           name="dist_ps")
                        for q in range(2):
                            c0 = h * HALF + q * 512
                            nc.tensor.matmul(
                                out=dist_ps[:, q * 512:(q + 1) * 512],
                                lhsT=phia_sb[:, i * P:(i + 1) * P],
                                rhs=phib_sb[:, c0:c0 + 512],
                                start=True, stop=True)
                        nc.vector.max(out=h16[:, h * 8:(h + 1) * 8], in_=dist_ps)
                    nc.vector.max(out=negd[:, i, :], in_=h16)

                # ---- stage 2: unpack idx / dw for the whole batch ----
                # packed = -(dist2 + m/2048); y = 2048*dist2 + m (exact int
                # in the region that matters); idx = y & 2047;
                # dist_norm = sqrt((y - idx) * 2^-25); dw = relu(R - dist_norm)
                y_f = epool.tile([P, nt, KNN], F32, tag="y_f", name="y_f")
                nc.vector.tensor_scalar(
                    out=y_f, in0=negd[:, :, 0:KNN], scalar1=-2048.0,
                    scalar2=None, op0=ALU.mult)
                y_u = epool.tile([P, nt, KNN], U32, tag="y_u", name="y_u")
                nc.vector.tensor_copy(out=y_u, in_=y_f)
                idx_u = epool.tile([P, nt, KNN], U32, tag="idx_u", name="idx_u")
                nc.vector.tensor_scalar(
                    out=idx_u, in0=y_u, scalar1=2047, scalar2=None,
                    op0=ALU.bitwise_and)
                d2_f = epool.tile([P, nt, KNN], F32, tag="d2_f", name="d2_f")
                nc.vector.tensor_tensor(out=d2_f, in0=y_u, in1=idx_u,
                                        op=ALU.subtract)
                dist4 = epool.tile([P, nt, KNN], F32, tag="dist4", name="dist4")
                nc.scalar.activation(out=dist4, in_=d2_f, func=AF.Sqrt,
                                     scale=float(2.0 ** -25))
                dw = epool.tile([P, nt, KNN], F32, tag="dw", name="dw")
                nc.scalar.activation(out=dw, in_=dist4, func=AF.Relu,
                                     scale=-1.0, bias=rconst)

                # ---- stage 3: Ya, gather, MLP combine ----
                for i in range(nt):
                    ya_ps = mmps.tile([P, D], F32, tag="mm", name="ya_ps")
                    for j in range(DT):
                        nc.tensor.matmul(
                            out=ya_ps, lhsT=fat_sb[:, j, i * P:(i + 1) * P],
                            rhs=w2_sb[:, j, :],
                            start=(j == 0), stop=(not with_bias and j == DT - 1))
                    if with_bias:
                        nc.tensor.matmul(out=ya_ps, lhsT=ones_t, rhs=bias_sb,
                                         start=False, stop=True)
                    ya_sb = iopool.tile([P, D], F16, tag="yasb", name="ya_sb")
                    nc.scalar.copy(out=ya_sb, in_=ya_ps)

                    ybg_k = []
                    for k in range(KNN):
                        ybg = gpool.tile([P, D], F16, tag=f"ybg{k}",
                                         name=f"ybg{k}")
                        nc.gpsimd.indirect_dma_start(
                            out=ybg[:],
                            out_offset=None,
                            in_=yb_dram[:],
                            in_offset=IndirectOffsetOnAxis(
                                ap=idx_u[:, i, k:k + 1], axis=0),
                        )
                        ybg_k.append(ybg)

                    r_k = []
                    for k in range(KNN):
                        zk = mpool.tile([P, D], F16, tag=f"z{k}", name=f"z{k}")
                        nc.vector.tensor_add(zk, ybg_k[k], ya_sb)
                        rk = mpool.tile([P, D], F16, tag=f"r{k}", name=f"r{k}")
                        if k < RELU_ON_ACT:
                            nc.scalar.activation(
                                out=rk, in_=zk, func=AF.Relu,
                                scale=dw[:, i, k:k + 1])
                        else:
                            nc.vector.tensor_scalar(
                                out=rk, in0=zk, scalar1=0.0,
                                scalar2=dw[:, i, k:k + 1],
                                op0=ALU.max, op1=ALU.mult)
                        r_k.append(rk)

                    s01 = mpool.tile([P, D], F16, tag="s01", name="s01")
                    nc.vector.tensor_add(s01, r_k[0], r_k[1])
                    s23 = mpool.tile([P, D], F16, tag="s23", name="s23")
                    nc.vector.tensor_add(s23, r_k[2], r_k[3])
                    fused = mpool.tile([P, D], F16, tag="fused", name="fused")
                    nc.vector.tensor_add(fused, s01, s23)
                    nc.sync.dma_start(out=out[b, i * P:(i + 1) * P, :], in_=fused)


# ---------------------------------------------------------------------------
# host side
# ---------------------------------------------------------------------------

def _host_inputs(feats_a, feats_b, W, bias, coords_a, coords_b):
    """Host-side prep: fp16 casts, chunk transposes, lifted packed coords."""
    d = W.shape[1]
    dt = d // P
    bsz, na_, _ = feats_a.shape
    nb_ = feats_b.shape[1]

    ca = coords_a.astype(np.int64)
    cb = coords_b.astype(np.int64)
    a2 = (ca * ca).sum(-1)                      # [B, Na] ints < 48388
    b2 = (cb * cb).sum(-1)
    hiA, loA = a2 >> 11, a2 & 2047
    hiB, loB = b2 >> 11, b2 & 2047
    ones = np.ones((bsz, na_), np.float32)
    m_over = (np.arange(nb_, dtype=np.float32) / 2048.0)[None, :].repeat(bsz, 0)
    # packed dot = 2a.b - |a|^2 - |b|^2 - m/2048 = -(dist2 + m/2048)
    phia8 = np.stack([ca[..., 0], ca[..., 1], ca[..., 2], hiA, loA,
                      2048 * np.ones((bsz, na_), np.int64),
                      np.ones((bsz, na_), np.int64),
                      np.ones((bsz, na_), np.int64)], axis=1).astype(np.float16)
    phib8 = np.stack([2.0 * cb[..., 0], 2.0 * cb[..., 1], 2.0 * cb[..., 2],
                      -2048 * np.ones((bsz, nb_), np.float64),
                      -np.ones((bsz, nb_), np.float64),
                      -hiB.astype(np.float64), -loB.astype(np.float64),
                      -m_over.astype(np.float64)], axis=1).astype(np.float16)

    # feats chunk-transposed: [B, dt, 128, N]
    fatT = np.ascontiguousarray(
        feats_a.reshape(bsz, na_, dt, P).transpose(0, 2, 3, 1)).astype(np.float16)
    fbtT = np.ascontiguousarray(
        feats_b.reshape(bsz, nb_, dt, P).transpose(0, 2, 3, 1)).astype(np.float16)

    w2f = W[d:]                                  # applies to a_f
    wdf = W[:d] - W[d:]                          # applies to b_f
    w2c = np.ascontiguousarray(w2f.reshape(dt, P, d)).astype(np.float16)
    wdc = np.ascontiguousarray(wdf.reshape(dt, P, d)).astype(np.float16)
    biasw = bias.reshape(1, d).astype(np.float16)
    return fatT, fbtT, phia8, phib8, w2c, wdc, biasw


def kernel(**inputs):
    feats_a = np.asarray(inputs["feats_a"], dtype=np.float32)
    feats_b = np.asarray(inputs["feats_b"], dtype=np.float32)
    W = np.asarray(inputs["W"], dtype=np.float32)
    bias = np.asarray(inputs["bias"], dtype=np.float32)
    coords_a = np.asarray(inputs["coords_a"])
    coords_b = np.asarray(inputs["coords_b"])

    fatT, fbtT, phia8, phib8, w2c, wdc, biasw = _host_inputs(
        feats_a, feats_b, W, bias, coords_a, coords_b)
    with_bias = bool(np.any(bias != 0.0))

    nc = build_bass(with_bias=with_bias)

    in_maps = []
    for c in range(N_CORES):
        s = slice(c * BLOC, (c + 1) * BLOC)
        in_maps.append({
            "fatT": np.ascontiguousarray(fatT[s]),
            "fbtT": np.ascontiguousarray(fbtT[s]),
            "phia": np.ascontiguousarray(phia8[s]),
            "phib": np.ascontiguousarray(phib8[s]),
            "w2": w2c,
            "wd": wdc,
            "biasw": biasw,
        })

    from concourse import bass_utils
    res = bass_utils.run_bass_kernel_spmd(nc, in_maps, core_ids=list(range(N_CORES)))
    fused = np.concatenate([r["out"] for r in res.results], axis=0)
    return np.concatenate([feats_a, fused.astype(np.float32)], axis=-1)


if __name__ == "__main__":
    nc = build_bass()
    print("built ok")
